# revision 7
# baseline (speedup 1.0000x reference)
"""Maxwell viscoelastic model (linear recurrence scan) on 8 Trainium2 NeuronCores.

Math (per trajectory, T timesteps):
    a_n = 1 - k*dt_n              (k = E/eta = 2)
    b_n = k*dt_n*eps_n
    gamma_n = a_n*gamma_{n-1} + b_n,  gamma_0 = 0
    sigma_n = (E_inf + E)*eps_n - E*gamma_n = 2.5*eps_n - 2*gamma_n

Kernel strategy: shard the batch (4096 trajectories) across 8 cores (512
each) — the recurrence is independent per trajectory, so pure data
parallelism.  Per core, 4 tiles of [128 partitions x 4096 timesteps].  The
recurrence runs on the DVE tensor_tensor_scan instruction:
    state = (data0 * state) + data1   per partition, along the free dim.
We scan g_n = a_n*g_{n-1} + (-E*b_n) so g = -E*gamma directly, then
sigma = (eps * 2.5) + g in one scalar_tensor_tensor op.

Raw bass (no TileContext): the Tile scheduler attaches semaphore waits
directly to instructions, and both the S2S2D2_STT struct (tensor_scalar /
stt / scan) and the tail Drain overflow their tiny ISA sync-wait budgets.
With raw bass every wait is its own instruction, and we control the
software pipeline explicitly:
  SYNC   loads xt tiles (HWDGE qSPDynamicHW ring)
  SCALAR computes a = 1 - k*dt (ACT) and issues output stores (qActDynamicHW)
  VECTOR computes bneg, the scan, and sigma
Double-buffered xt/a/sig; bneg/g single-buffered (DVE-serial).
"""

import numpy as np

import concourse.bass as bass
import concourse.mybir as mybir
from concourse.bass_utils import run_bass_kernel_spmd

E = 2.0
ETA = 1.0
E_INFTY = 0.5
K = E / ETA                  # 2.0
NEG_EK = -(E * K)            # -4.0: scan data1 scale so the scan outputs -E*gamma
SIG_EPS = E_INFTY + E        # 2.5

N_CORES = 8
P = 128                      # SBUF partitions


def build_nc(b_shard: int, t_len: int) -> bass.Bass:
    nc = bass.Bass()
    x = nc.dram_tensor("x", [b_shard, t_len, 2], mybir.dt.float32, kind="ExternalInput")
    y = nc.dram_tensor("y", [b_shard, t_len], mybir.dt.float32, kind="ExternalOutput")
    n_tiles = b_shard // P
    assert n_tiles * P == b_shard

    xr = x.rearrange("(n p) t c -> n p t c", p=P)   # [n_tiles, 128, T, 2]
    yr = y.rearrange("(n p) t -> n p t", p=P)       # [n_tiles, 128, T]
    f32 = mybir.dt.float32
    mult = mybir.AluOpType.mult
    add = mybir.AluOpType.add

    with (
        nc.sbuf_tensor("xt0", [P, t_len, 2], f32) as xt0,
        nc.sbuf_tensor("xt1", [P, t_len, 2], f32) as xt1,
        nc.sbuf_tensor("a0", [P, t_len], f32) as a0,
        nc.sbuf_tensor("a1", [P, t_len], f32) as a1,
        nc.sbuf_tensor("bneg", [P, t_len], f32) as bneg,
        nc.sbuf_tensor("g", [P, t_len], f32) as g,
        nc.sbuf_tensor("sig0", [P, t_len], f32) as sig0,
        nc.sbuf_tensor("sig1", [P, t_len], f32) as sig1,
        nc.semaphore("dma_in0") as dma_in0,    # +16 per even xt tile load
        nc.semaphore("dma_in1") as dma_in1,    # +16 per odd xt tile load
        nc.semaphore("dma_out0") as dma_out0,  # +16 per even sigma store
        nc.semaphore("dma_out1") as dma_out1,  # +16 per odd sigma store
        nc.semaphore("act_a") as act_a,        # +1 per a tile (ACT)
        nc.semaphore("dve_seq") as dve_seq,    # +1 per DVE instruction
        nc.Block() as block,
    ):
        xt = [xt0, xt1]
        a = [a0, a1]
        sig = [sig0, sig1]
        # Ping/pong DMA semaphores: two DMAs on one ring can complete out of
        # order, so a shared counter cannot tell which transfer finished.
        dma_in = [dma_in0, dma_in1]
        dma_out = [dma_out0, dma_out1]
        # DVE emits 3 instructions per tile; dve_seq after tile i's
        # bneg/scan/sigma is 3i+1 / 3i+2 / 3i+3.

        @block.sync
        def _(sync):
            for i in range(n_tiles):
                if i >= 2:
                    # xt slot reuse: sigma of tile i-2 is the last reader.
                    sync.wait_ge(dve_seq, 3 * (i - 2) + 3)
                sync.dma_start(xt[i % 2][:], xr[i]).then_inc(dma_in[i % 2], 16)

        @block.scalar
        def _(scalar):
            def store(k):
                scalar.wait_ge(dve_seq, 3 * k + 3)   # sigma_k complete
                scalar.dma_start(yr[k], sig[k % 2][:]).then_inc(dma_out[k % 2], 16)

            for i in range(n_tiles):
                if i >= 2:
                    store(i - 2)
                scalar.wait_ge(dma_in[i % 2], 16 * (i // 2 + 1))
                if i >= 2:
                    # a slot reuse: scan of tile i-2 read it.
                    scalar.wait_ge(dve_seq, 3 * (i - 2) + 2)
                # a = Copy(dt * -K + 1)
                scalar.activation(
                    a[i % 2][:], xt[i % 2][:, :, 1],
                    mybir.ActivationFunctionType.Copy,
                    bias=1.0, scale=-K,
                ).then_inc(act_a, 1)
            for k in range(max(0, n_tiles - 2), n_tiles):
                store(k)
            scalar.wait_ge(dma_out0, 16 * ((n_tiles + 1) // 2))
            if n_tiles >= 2:
                scalar.wait_ge(dma_out1, 16 * (n_tiles // 2))

        @block.vector
        def _(vector):
            # The DVE pipelines back-to-back instructions, so same-engine
            # RAW/WAR hazards need semaphore sync (each instruction incs
            # dve_seq on completion; dependents wait for it).
            for i in range(n_tiles):
                eps = xt[i % 2][:, :, 0]
                dtv = xt[i % 2][:, :, 1]
                vector.wait_ge(dma_in[i % 2], 16 * (i // 2 + 1))
                if i >= 1:
                    # bneg WAR: scan of tile i-1 was the last bneg reader.
                    vector.wait_ge(dve_seq, 3 * (i - 1) + 2)
                # bneg = (dt * -E*K) * eps
                vector.scalar_tensor_tensor(
                    bneg[:], dtv, NEG_EK, eps, mult, mult,
                ).then_inc(dve_seq, 1)
                vector.wait_ge(act_a, i + 1)
                vector.wait_ge(dve_seq, 3 * i + 1)   # bneg complete
                # g_n = a_n*g_{n-1} + bneg_n  ->  g = -E*gamma
                vector.tensor_tensor_scan(
                    g[:], a[i % 2][:], bneg[:], 0.0, mult, add,
                ).then_inc(dve_seq, 1)
                if i >= 2:
                    # sig slot reuse: store of tile i-2 must have completed.
                    vector.wait_ge(dma_out[i % 2], 16 * ((i - 2) // 2 + 1))
                vector.wait_ge(dve_seq, 3 * i + 2)   # scan complete
                # sigma = (eps * 2.5) + g
                vector.scalar_tensor_tensor(
                    sig[i % 2][:], eps, SIG_EPS, g[:], mult, add,
                ).then_inc(dve_seq, 1)

    return nc


_NC_CACHE: dict = {}


def _get_nc(b_shard: int, t_len: int) -> bass.Bass:
    key = (b_shard, t_len)
    if key not in _NC_CACHE:
        _NC_CACHE[key] = build_nc(b_shard, t_len)
    return _NC_CACHE[key]


def run(x: np.ndarray, trace: bool = False):
    """Run the sharded kernel; returns (full_output, BassKernelResults)."""
    b, t_len, c = x.shape
    assert c == 2 and b % N_CORES == 0
    b_shard = b // N_CORES
    x = np.ascontiguousarray(np.asarray(x, dtype=np.float32))
    shards = x.reshape(N_CORES, b_shard, t_len, 2)
    in_maps = [{"x": shards[i]} for i in range(N_CORES)]
    res = run_bass_kernel_spmd(
        _get_nc(b_shard, t_len), in_maps,
        core_ids=list(range(N_CORES)), trace=trace,
    )
    out = np.concatenate([r["y"] for r in res.results], axis=0)
    return out.reshape(b, t_len, 1), res


def kernel(x: np.ndarray) -> np.ndarray:
    out, _ = run(x, trace=False)
    return out


# revision 8
# speedup vs baseline: 1.0865x; 1.0865x over previous
"""Maxwell viscoelastic model (linear recurrence scan) on 8 Trainium2 NeuronCores.

Math (per trajectory, T timesteps):
    a_n = 1 - k*dt_n              (k = E/eta = 2)
    b_n = k*dt_n*eps_n
    gamma_n = a_n*gamma_{n-1} + b_n,  gamma_0 = 0
    sigma_n = (E_inf + E)*eps_n - E*gamma_n = 2.5*eps_n - 2*gamma_n

Kernel strategy: shard the batch (4096 trajectories) across 8 cores (512
each) — the recurrence is independent per trajectory, so pure data
parallelism.  Per core, 4 tiles of [128 partitions x 4096 timesteps].  The
recurrence runs on the DVE tensor_tensor_scan instruction:
    state = (data0 * state) + data1   per partition, along the free dim.
We scan g_n = a_n*g_{n-1} + (-E*b_n) so g = -E*gamma directly, then
sigma = (eps * 2.5) + g in one scalar_tensor_tensor op.

Raw bass (no TileContext): the Tile scheduler attaches semaphore waits
directly to instructions, and both the S2S2D2_STT struct (tensor_scalar /
stt / scan) and the tail Drain overflow their tiny ISA sync-wait budgets.
With raw bass every wait is its own instruction, and we control the
software pipeline explicitly:
  SYNC   loads xt tiles (HWDGE qSPDynamicHW ring)
  SCALAR computes a = 1 - k*dt (ACT) and issues output stores (qActDynamicHW)
  VECTOR computes bneg, the scan, and sigma
Double-buffered xt/a/sig; bneg/g single-buffered (DVE-serial).
"""

import numpy as np

import concourse.bass as bass
import concourse.mybir as mybir
from concourse.bass_utils import run_bass_kernel_spmd

E = 2.0
ETA = 1.0
E_INFTY = 0.5
K = E / ETA                  # 2.0
NEG_EK = -(E * K)            # -4.0: scan data1 scale so the scan outputs -E*gamma
SIG_EPS = E_INFTY + E        # 2.5

N_CORES = 8
P = 128                      # SBUF partitions


CH = 4                       # chunks along the time axis per tile


def build_nc(b_shard: int, t_len: int) -> bass.Bass:
    """Chunked software pipeline.

    Per tile (128 trajectories x T), the time axis is cut into CH chunks so
    loads/compute/stores stream at chunk granularity: compute starts after
    the first 1/CH of the first load, and the final store is only 1/CH of a
    tile. The scan chains across chunks via initial=g[:, chunk_start-1].
    Global chunk index q = CH*i + c; DVE emits 3 instructions per chunk
    (bneg/scan/sigma at dve_seq 3q+1 / 3q+2 / 3q+3).
    """
    nc = bass.Bass()
    x = nc.dram_tensor("x", [b_shard, t_len, 2], mybir.dt.float32, kind="ExternalInput")
    y = nc.dram_tensor("y", [b_shard, t_len], mybir.dt.float32, kind="ExternalOutput")
    n_tiles = b_shard // P
    assert n_tiles * P == b_shard
    assert t_len % CH == 0
    L = t_len // CH

    xr = x.rearrange("(n p) t c -> n p t c", p=P)   # [n_tiles, 128, T, 2]
    yr = y.rearrange("(n p) t -> n p t", p=P)       # [n_tiles, 128, T]
    f32 = mybir.dt.float32
    mult = mybir.AluOpType.mult
    add = mybir.AluOpType.add

    def cs(c):
        return slice(c * L, (c + 1) * L)

    with (
        nc.sbuf_tensor("xt0", [P, t_len, 2], f32) as xt0,
        nc.sbuf_tensor("xt1", [P, t_len, 2], f32) as xt1,
        nc.sbuf_tensor("a0", [P, t_len], f32) as a0,
        nc.sbuf_tensor("a1", [P, t_len], f32) as a1,
        nc.sbuf_tensor("bneg", [P, L], f32) as bneg,
        nc.sbuf_tensor("g", [P, t_len], f32) as g,
        nc.sbuf_tensor("sig0", [P, t_len], f32) as sig0,
        nc.sbuf_tensor("sig1", [P, t_len], f32) as sig1,
        nc.semaphore("act_a") as act_a,        # +1 per a chunk (ACT)
        nc.semaphore("dve_seq") as dve_seq,    # +1 per DVE instruction
        nc.Block(no_gpsimd_drain=True) as block,
    ):
        # Per (slot, chunk) DMA semaphores: DMAs complete out of order, so
        # each (buffer slot, chunk) needs its own completion counter.
        sem_in = [[nc.alloc_semaphore(f"in{s}_{c}") for c in range(CH)] for s in range(2)]
        sem_out = [[nc.alloc_semaphore(f"out{s}_{c}") for c in range(CH)] for s in range(2)]
        xt = [xt0, xt1]
        a = [a0, a1]
        sig = [sig0, sig1]

        @block.sync
        def _(sync):
            for i in range(n_tiles):
                for c in range(CH):
                    if i >= 2:
                        # xt slot chunk reuse: sigma(i-2, c) was the last reader.
                        sync.wait_ge(dve_seq, 3 * (CH * (i - 2) + c) + 3)
                    sync.dma_start(
                        xt[i % 2][:, cs(c), :], xr[i][:, cs(c), :]
                    ).then_inc(sem_in[i % 2][c], 16)

        @block.scalar
        def _(scalar):
            def store(k):
                i, c = divmod(k, CH)
                scalar.wait_ge(dve_seq, 3 * k + 3)   # sigma(k) complete
                scalar.dma_start(
                    yr[i][:, cs(c)], sig[i % 2][:, cs(c)]
                ).then_inc(sem_out[i % 2][c], 16)

            for i in range(n_tiles):
                for c in range(CH):
                    q = CH * i + c
                    scalar.wait_ge(sem_in[i % 2][c], 16 * (i // 2 + 1))
                    if i >= 2:
                        # a slot chunk reuse: scan(i-2, c) read it.
                        scalar.wait_ge(dve_seq, 3 * (CH * (i - 2) + c) + 2)
                    # a = Copy(dt * -K + 1)
                    scalar.activation(
                        a[i % 2][:, cs(c)], xt[i % 2][:, cs(c), 1],
                        mybir.ActivationFunctionType.Copy,
                        bias=1.0, scale=-K,
                    ).then_inc(act_a, 1)
                    if q >= 1:
                        store(q - 1)
            store(CH * n_tiles - 1)
            for c in range(CH):
                scalar.wait_ge(sem_out[0][c], 16 * ((n_tiles + 1) // 2))
                if n_tiles >= 2:
                    scalar.wait_ge(sem_out[1][c], 16 * (n_tiles // 2))

        @block.vector
        def _(vector):
            # The DVE pipelines back-to-back instructions, so same-engine
            # RAW/WAR hazards need semaphore sync (each instruction incs
            # dve_seq on completion; dependents wait for it).
            for i in range(n_tiles):
                for c in range(CH):
                    q = CH * i + c
                    eps = xt[i % 2][:, cs(c), 0]
                    dtv = xt[i % 2][:, cs(c), 1]
                    vector.wait_ge(sem_in[i % 2][c], 16 * (i // 2 + 1))
                    if q >= 1:
                        # bneg WAR: scan(q-1) was the last bneg reader.
                        vector.wait_ge(dve_seq, 3 * (q - 1) + 2)
                    # bneg = (dt * -E*K) * eps
                    vector.scalar_tensor_tensor(
                        bneg[:], dtv, NEG_EK, eps, mult, mult,
                    ).then_inc(dve_seq, 1)
                    vector.wait_ge(act_a, q + 1)
                    vector.wait_ge(dve_seq, 3 * q + 1)   # bneg complete
                    # g_n = a_n*g_{n-1} + bneg_n  ->  g = -E*gamma
                    # Chain across chunks: initial = last element of the
                    # previous chunk (same tile); fresh 0 at chunk 0.
                    init = 0.0 if c == 0 else g[:, c * L - 1:c * L]
                    vector.tensor_tensor_scan(
                        g[:, cs(c)], a[i % 2][:, cs(c)], bneg[:], init, mult, add,
                    ).then_inc(dve_seq, 1)
                    if i >= 2:
                        # sig slot chunk reuse: store(i-2, c) completed.
                        vector.wait_ge(sem_out[i % 2][c], 16 * ((i - 2) // 2 + 1))
                    vector.wait_ge(dve_seq, 3 * q + 2)   # scan complete
                    # sigma = (eps * 2.5) + g
                    vector.scalar_tensor_tensor(
                        sig[i % 2][:, cs(c)], eps, SIG_EPS, g[:, cs(c)], mult, add,
                    ).then_inc(dve_seq, 1)

    return nc


_NC_CACHE: dict = {}


def _get_nc(b_shard: int, t_len: int) -> bass.Bass:
    key = (b_shard, t_len)
    if key not in _NC_CACHE:
        _NC_CACHE[key] = build_nc(b_shard, t_len)
    return _NC_CACHE[key]


def run(x: np.ndarray, trace: bool = False):
    """Run the sharded kernel; returns (full_output, BassKernelResults)."""
    b, t_len, c = x.shape
    assert c == 2 and b % N_CORES == 0
    b_shard = b // N_CORES
    x = np.ascontiguousarray(np.asarray(x, dtype=np.float32))
    shards = x.reshape(N_CORES, b_shard, t_len, 2)
    in_maps = [{"x": shards[i]} for i in range(N_CORES)]
    res = run_bass_kernel_spmd(
        _get_nc(b_shard, t_len), in_maps,
        core_ids=list(range(N_CORES)), trace=trace,
    )
    out = np.concatenate([r["y"] for r in res.results], axis=0)
    return out.reshape(b, t_len, 1), res


def kernel(x: np.ndarray) -> np.ndarray:
    out, _ = run(x, trace=False)
    return out
